# revision 1
# baseline (speedup 1.0000x reference)
"""Trainium2 Bass kernel for the KalmanFilter linear recurrence.

  x = data - mean;  z0 = R @ x[0];  drive = inputs @ C.T
  z_{t+1} = A z_t + drive[t]   (T = 32768 steps, dim 512)
  result  = Z[1:] @ B.T + mean

Strategy (8 NeuronCores, sequence-parallel):
  - ||A^k|| decays like 0.9^k (spectral radius 0.9), so the recurrence
    forgets its state after H=128 steps to ~1e-5 relative.
  - Each core owns 4096 contiguous steps, split into 256 chunks of S=16
    steps + K=8 extra "halo" chunks covering the preceding H=128 steps.
  - Phase A: batched zero-init scan over all 268 chunks (state tiles
    [512, 268], 15 matmul steps) -> per-chunk accumulated drives b_c.
  - Phase B: chunk-start states w_c = sum_{p=0}^{K-1} (A^16)^p b_{c-1-p}
    (banded combine truncated at ||A^128|| ~ 4e-4 of a unit). The tap
    matrices (A^16)^p are computed ON DEVICE by repeated squaring in TF32.
  - Phase C: re-scan the 256 real chunks from inits w_c; each step also
    applies the output projection B.T and streams fp16 rows to DRAM.
  - z0 only affects output rows 0..H-1 (through A^n z0); that correction
    and the +mean are added on the host, overlapped with the download.

The end-to-end time is dominated by the ~45 MB/s per-process axon
host<->device pipe, so the wire format is minimal:
  - u is shipped pre-transposed in fp16 (2.2 MB/core incl halo);
  - A.T / C.T / B.T / identity are shared by all cores: each core uploads
    only a 1/8 slice and an on-device AllGather collective rebuilds them
    (1.8 MB total instead of 14.5 MB);
  - the output returns as fp16 (33.5 MB); +mean happens on the host;
  - the PJRT output buffers are created on device once and reused
    WITHOUT donation -- the kernel writes every output element, so stale
    contents are harmless and no host zeros ever cross the link.
Total wire traffic ~54 MB vs ~220 MB for the naive path.  (A
multi-process split was tried -- per-process tunnels do scale to
~240 MB/s aggregate -- but this box has 1 CPU and eight jax clients
thrash it; single-process is faster end-to-end.)
"""
import numpy as np
import concourse.bacc as bacc
import concourse.mybir as mybir
from concourse import tile

T = 32768
DZ = 512
DU = 256
NCORE = 8
TLOC = T // NCORE          # 4096
S = 16                     # steps per chunk
BCH = TLOC // S            # 256 chunks per core
H = 128                    # halo steps (forgetting horizon)
K = H // S                 # 8 banded taps (incl. identity)
NCH = BCH + K              # 268 chunks in phase A
ULEN = TLOC + H            # 4288 drive rows per core
UPAD = ((ULEN + 127) // 128) * 128   # 4352, padded to a multiple of 128

f32 = mybir.dt.float32
f32r = mybir.dt.float32r
fp16 = mybir.dt.float16

_CACHE = {}


def _emit(nc):
    ut_d = nc.dram_tensor("ut", (DU, UPAD), fp16, kind="ExternalInput")
    at_d = nc.dram_tensor("at", (DZ // NCORE, DZ), f32r, kind="ExternalInput")
    ct_d = nc.dram_tensor("ct", (DU // NCORE, DZ), fp16, kind="ExternalInput")
    bt_d = nc.dram_tensor("bt", (DZ // NCORE, DZ), fp16, kind="ExternalInput")
    id_d = nc.dram_tensor("id", (128 // NCORE, 128), f32, kind="ExternalInput")
    out_d = nc.dram_tensor("out", (TLOC, DZ), fp16, kind="ExternalOutput")

    with tile.TileContext(nc) as tc:
        with tc.tile_pool(name="const", bufs=1) as cpool, \
             tc.tile_pool(name="dt", bufs=1) as dpool, \
             tc.tile_pool(name="st", bufs=2) as stpool, \
             tc.tile_pool(name="ob", bufs=4) as opool, \
             tc.tile_pool(name="dram", bufs=1, space="DRAM") as drampool, \
             tc.tile_pool(name="ps", bufs=8, space="PSUM") as pp:

            # ---- AllGather shared constants (each core ships 1/8) ----
            rg = [list(range(NCORE))]
            byp = mybir.AluOpType.bypass
            ag = {}
            for nm, dram_in, shape, dty in (
                    ("at", at_d, (DZ, DZ), f32r),
                    ("ct", ct_d, (DU, DZ), fp16),
                    ("bt", bt_d, (DZ, DZ), fp16),
                    ("id", id_d, (128, 128), f32)):
                bi = drampool.tile([shape[0] // NCORE, shape[1]], dty,
                                   tag=f"agi_{nm}", name=f"agi_{nm}")
                bo = drampool.tile(list(shape), dty, tag=f"ago_{nm}",
                                   name=f"ago_{nm}")
                nc.gpsimd.dma_start(bi[:], dram_in[:])
                nc.gpsimd.collective_compute(
                    "AllGather", byp, replica_groups=rg,
                    ins=[bi.opt()], outs=[bo.opt()])
                ag[nm] = bo

            # ---- constant loads ----
            at_sb = [cpool.tile([128, DZ], f32r, tag=f"at{k}", name=f"at{k}") for k in range(4)]
            ct_sb = [cpool.tile([128, DZ], fp16, tag=f"ct{k}", name=f"ct{k}") for k in range(2)]
            bth = [cpool.tile([128, DZ], fp16, tag=f"bth{k}", name=f"bth{k}") for k in range(4)]
            bt_sb = [cpool.tile([128, DZ], f32r, tag=f"bt{k}", name=f"bt{k}") for k in range(4)]
            id_sb = cpool.tile([128, 128], f32, tag="id")
            ut_sb = [dpool.tile([128, UPAD], fp16, tag=f"ut{k}", name=f"ut{k}") for k in range(2)]
            for k in range(4):
                nc.sync.dma_start(at_sb[k][:], ag["at"][128 * k:128 * (k + 1), :])
                nc.sync.dma_start(bth[k][:], ag["bt"][128 * k:128 * (k + 1), :])
            for k in range(2):
                nc.sync.dma_start(ct_sb[k][:], ag["ct"][128 * k:128 * (k + 1), :])
                nc.sync.dma_start(ut_sb[k][:], ut_d[128 * k:128 * (k + 1), :])
            nc.sync.dma_start(id_sb[:], ag["id"][:])
            for k in range(4):
                nc.vector.tensor_copy(bt_sb[k][:], bth[k][:])   # fp16 -> f32

            # drive rows (transposed): dt[m] holds drive.T[128m:128(m+1), :]
            dt_sb = [dpool.tile([128, UPAD], f32, tag=f"dt{m}", name=f"dt{m}") for m in range(4)]
            for nb in range((UPAD + 511) // 512):
                nb0 = nb * 512
                w = min(512, UPAD - nb0)
                for m in range(4):
                    psd = pp.tile([128, 512], f32, tag="ps", name=f"psD{nb}_{m}")
                    for kk in range(2):
                        nc.tensor.matmul(
                            psd[:, :w],
                            ct_sb[kk][:, 128 * m:128 * (m + 1)],
                            ut_sb[kk][:, nb0:nb0 + w],
                            start=(kk == 0), stop=(kk == 1))
                    nc.any.tensor_copy(dt_sb[m][:, nb0:nb0 + w], psd[:, :w])

            # ---- phase A: zero-init scan over NCH chunks ----
            bmat = [cpool.tile([128, NCH], f32r, tag=f"bm{m}", name=f"bm{m}") for m in range(4)]
            st_prev = []
            for m in range(4):
                t0 = stpool.tile([128, NCH], f32r, tag=f"st{m}", name=f"st0_{m}")
                nc.vector.tensor_copy(t0[:], dt_sb[m][:, 0:16 * NCH:16])
                st_prev.append(t0)
            for k in range(1, S):
                psl = [pp.tile([128, NCH], f32, tag="ps", name=f"psA{k}_{_m}") for _m in range(4)]
                for m in range(4):
                    for kk in range(4):
                        nc.tensor.matmul(
                            psl[m][:],
                            at_sb[kk][:, 128 * m:128 * (m + 1)],
                            st_prev[kk][:],
                            start=(kk == 0), stop=(kk == 3))
                st_new = []
                for m in range(4):
                    dst = (bmat[m] if k == S - 1 else
                           stpool.tile([128, NCH], f32r, tag=f"st{m}", name=f"stA{k}_{m}"))
                    nc.vector.tensor_tensor(
                        dst[:], psl[m][:],
                        dt_sb[m][:, k:k + 16 * (NCH - 1) + 1:16],
                        op=mybir.AluOpType.add)
                    st_new.append(dst)
                st_prev = st_new

            # ---- device-side tap matrices: G^(16p), G = A.T, via squaring ----
            def mat_t(src, dst, tg):      # dst = src.T
                for k in range(4):
                    for m in range(4):
                        pst = pp.tile([128, 128], f32, tag="ps", name=f"pT{tg}_{k}_{m}")
                        nc.tensor.transpose(
                            pst[:], src[k][:, 128 * m:128 * (m + 1)].bitcast(f32), id_sb[:])
                        nc.any.tensor_copy(dst[m][:, 128 * k:128 * (k + 1)], pst[:])

            def mat_mul(xT, y, dst, tg):  # dst = X @ Y  (xT = row-tiles of X.T)
                for m in range(4):
                    ps = pp.tile([128, DZ], f32, tag="ps", name=f"pM{tg}_{m}")
                    for k in range(4):
                        nc.tensor.matmul(
                            ps[:],
                            xT[k][:, 128 * m:128 * (m + 1)],
                            y[k][:],
                            start=(k == 0), stop=(k == 3))
                    nc.any.tensor_copy(dst[m][:], ps[:])

            # three rotating 512x512 buffers: px = transpose scratch,
            # py = current power, pz = G^16 (after the squaring chain)
            px = [cpool.tile([128, DZ], f32r, tag=f"px{m}", name=f"px{m}") for m in range(4)]
            py = [cpool.tile([128, DZ], f32r, tag=f"py{m}", name=f"py{m}") for m in range(4)]
            pz = [cpool.tile([128, DZ], f32r, tag=f"pz{m}", name=f"pz{m}") for m in range(4)]

            mat_t(at_sb, px, "a")          # px = A row-tiles (= G.T)
            mat_mul(px, at_sb, py, "g2")   # py = G^2
            mat_t(py, px, "t2")
            mat_mul(px, py, pz, "g4")      # pz = G^4
            mat_t(pz, px, "t4")
            mat_mul(px, pz, py, "g8")      # py = G^8
            mat_t(py, px, "t8")
            mat_mul(px, py, pz, "g16")     # pz = G^16 (kept for the chain)

            # ---- phase B: banded combine  w_c = sum_p (A^16)^p b_{c-1-p} ----
            w_prev = []
            for m in range(4):
                wt = stpool.tile([128, BCH], f32r, tag=f"w{m}", name=f"w0_{m}")
                nc.vector.tensor_copy(wt[:], bmat[m][:, K - 1:K - 1 + BCH].bitcast(f32))
                w_prev.append(wt)
            pcur = pz
            for p in range(1, K):
                if p > 1:
                    mat_t(pcur, px, f"tp{p}")
                    mat_mul(px, pz, py, f"pp{p}")
                    pcur = py
                lo = K - 1 - p
                w_new = []
                for m in range(4):
                    ps = pp.tile([128, BCH], f32, tag="ps", name=f"psW{p}_{m}")
                    for kk in range(4):
                        nc.tensor.matmul(
                            ps[:],
                            pcur[kk][:, 128 * m:128 * (m + 1)],
                            bmat[kk][:, lo:lo + BCH],
                            start=(kk == 0), stop=(kk == 3))
                    wt = stpool.tile([128, BCH], f32r, tag=f"w{m}", name=f"w{p}_{m}")
                    nc.vector.tensor_tensor(
                        wt[:], w_prev[m][:].bitcast(f32), ps[:], op=mybir.AluOpType.add)
                    w_new.append(wt)
                w_prev = w_new

            # ---- phase C: scan 256 chunks from w_c, fused output proj ----
            st_prev = w_prev
            for k in range(S):
                psl = [pp.tile([128, BCH], f32, tag="ps", name=f"psC{k}_{_m}") for _m in range(4)]
                for m in range(4):
                    for kk in range(4):
                        nc.tensor.matmul(
                            psl[m][:],
                            at_sb[kk][:, 128 * m:128 * (m + 1)],
                            st_prev[kk][:],
                            start=(kk == 0), stop=(kk == 3))
                st_new = []
                for m in range(4):
                    dst = stpool.tile([128, BCH], f32r, tag=f"sc{m}", name=f"stC{k}_{m}")
                    nc.vector.tensor_tensor(
                        dst[:], psl[m][:],
                        dt_sb[m][:, H + k:H + k + 16 * (BCH - 1) + 1:16],
                        op=mybir.AluOpType.add)
                    st_new.append(dst)
                st_prev = st_new
                # output rows t = 16*c + k for all 256 chunks c
                for h in range(2):
                    pso = pp.tile([128, DZ], f32, tag="ps", name=f"psO{k}_{h}")
                    for kk in range(4):
                        nc.tensor.matmul(
                            pso[:],
                            st_new[kk][:, 128 * h:128 * (h + 1)],
                            bt_sb[kk][:],
                            start=(kk == 0), stop=(kk == 3))
                    ob = opool.tile([128, DZ], fp16, tag="ob", name=f"ob{k}_{h}")
                    nc.any.tensor_copy(ob[:], pso[:])
                    r0 = 2048 * h + k
                    nc.sync.dma_start(out_d[r0:r0 + 2033:16, :], ob[:])
    nc.compile()
    return nc


def _build():
    if "nc" not in _CACHE:
        nc = bacc.Bacc("TRN2", target_bir_lowering=False, debug=False,
                       num_devices=NCORE)
        _CACHE["nc"] = _emit(nc)
    return _CACHE["nc"]


def _make_exec(nc):
    """Minimal replication of run_bass_via_pjrt. The output buffers are
    created on device ONCE and reused without donation -- the kernel writes
    every output element, so no host zeros ever cross the tunnel."""
    import functools
    import jax
    import jax.numpy as jnp
    from jax.sharding import Mesh, PartitionSpec, NamedSharding
    from jax.experimental.shard_map import shard_map
    from concourse import bass2jax as b2j

    b2j.install_neuronx_cc_hook()

    partition_name = nc.partition_id_tensor.name if nc.partition_id_tensor else None
    in_names, out_names, out_avals = [], [], []
    for alloc in nc.m.functions[0].allocations:
        if not isinstance(alloc, mybir.MemoryLocationSet):
            continue
        name = alloc.memorylocations[0].name
        if alloc.kind == "ExternalInput":
            if name != partition_name:
                in_names.append(name)
        elif alloc.kind == "ExternalOutput":
            shape = tuple(alloc.tensor_shape)
            dtype = mybir.dt.np(alloc.dtype)
            out_names.append(name)
            out_avals.append(jax.core.ShapedArray(shape, dtype))
    n_params = len(in_names)
    all_in = tuple(in_names + out_names + ([partition_name] if partition_name else []))

    def _body(*args):
        operands = list(args)
        if partition_name:
            operands.append(b2j.partition_id_tensor())
        outs = b2j._bass_exec_p.bind(
            *operands,
            out_avals=tuple(out_avals),
            in_names=all_in,
            out_names=tuple(out_names),
            lowering_input_output_aliases=(),
            sim_require_finite=True,
            sim_require_nnan=True,
            nc=nc,
        )
        return tuple(outs)

    devices = jax.devices()[:NCORE]
    mesh = Mesh(np.asarray(devices), ("core",))
    sharded = jax.jit(
        shard_map(
            _body, mesh=mesh,
            in_specs=(PartitionSpec("core"),) * (n_params + len(out_names)),
            out_specs=(PartitionSpec("core"),) * len(out_names),
            check_rep=False),
        keep_unused=True)

    shd = NamedSharding(mesh, PartitionSpec("core"))
    obufs = [
        jax.jit(functools.partial(
            jnp.zeros, (NCORE * a.shape[0],) + tuple(a.shape[1:]), a.dtype),
            out_shardings=shd)()
        for a in out_avals
    ]
    dbg_name = nc.dbg_addr.name if nc.dbg_addr is not None else None
    return {"sharded": sharded, "in_names": in_names, "out_names": out_names,
            "out_avals": out_avals, "obufs": obufs, "dbg_name": dbg_name}


def _get_state():
    if "exec" not in _CACHE:
        _CACHE["exec"] = _make_exec(_build())
    return _CACHE["exec"]


def _host_prep(inputs_np, A, B, C):
    """Per-run host prep: global (concatenated-over-cores) input arrays.
    The shared constants are shipped once (1/8 slice per core + on-device
    AllGather), so no np.tile here."""
    upT16 = np.ascontiguousarray(inputs_np.T).astype(np.float16)   # (DU, T)
    ut_g = np.zeros((NCORE * DU, UPAD), np.float16)
    for i in range(NCORE):
        if i == 0:
            ut_g[:DU, H:ULEN] = upT16[:, :TLOC]
        else:
            lo = i * TLOC - H
            ut_g[i * DU:(i + 1) * DU, :ULEN] = upT16[:, lo:lo + TLOC + H]
    feed = {
        "ut": ut_g,
        "at": np.ascontiguousarray(A.T, dtype=np.float32),
        "ct": np.ascontiguousarray(C.T).astype(np.float16),
        "bt": np.ascontiguousarray(B.T).astype(np.float16),
        "id": np.eye(128, dtype=np.float32),
    }
    return feed


def _correction(data, mean, A, B, R):
    """Output rows 0..H-1 need the A^n z0 contribution."""
    z0 = R.astype(np.float64) @ (data[0] - mean[0]).astype(np.float64)
    zc = z0
    A64, B64 = A.astype(np.float64), B.astype(np.float64)
    corr = np.empty((H, DZ), np.float64)
    for n in range(1, H + 1):
        zc = A64 @ zc
        corr[n - 1] = B64 @ zc
    return corr.astype(np.float32)


def _invoke(state, data, inputs_np, mean, A, B, C, R):
    """One full run: prep, upload + execute, then overlap the z0-correction
    math with the output download."""
    feed = _host_prep(inputs_np, A, B, C)
    if state["dbg_name"] is not None:
        feed = {**feed, state["dbg_name"]: np.zeros((NCORE, 2), np.uint32)}
    args = [feed[n] for n in state["in_names"]]
    outs = state["sharded"](*args, *state["obufs"])
    try:
        outs[0].copy_to_host_async()
    except Exception:
        pass
    corr = _correction(data, mean, A, B, R)
    o16 = np.asarray(outs[0])                 # (T, DZ) fp16
    out = np.add(o16, mean, dtype=np.float32)
    out[:H] += corr
    return out


def kernel(data, inputs, mean, A, B, C, recognition_matrix, steps=None, **kw):
    data = np.asarray(data, np.float32)
    inputs_np = np.asarray(inputs, np.float32)
    mean = np.asarray(mean, np.float32)
    A = np.asarray(A, np.float32)
    B = np.asarray(B, np.float32)
    C = np.asarray(C, np.float32)
    R = np.asarray(recognition_matrix, np.float32)

    state = _get_state()
    return _invoke(state, data, inputs_np, mean, A, B, C, R)



# revision 3
# speedup vs baseline: 1.6484x; 1.6484x over previous
"""Trainium2 Bass kernel for the KalmanFilter linear recurrence.

  x = data - mean;  z0 = R @ x[0];  drive = inputs @ C.T
  z_{t+1} = A z_t + drive[t]   (T = 32768 steps, dim 512)
  result  = Z[1:] @ B.T + mean

Strategy (8 NeuronCores, sequence-parallel + host<->device pipelining):
  - ||A^k|| decays like 0.9^k (spectral radius 0.9), so the recurrence
    forgets its state after H=128 steps to ~1e-5 relative.  Each core
    owns 4096 contiguous steps; those are processed independently per
    128-step-halo'd chunk exactly as in the classic blocked scan:
      Phase A: zero-init scan over 16-step chunks -> accumulated drives
      Phase B: banded combine with on-device (A^16)^p tap matrices
      Phase C: re-scan from combined inits, fused output projection B.T
  - The end-to-end time is dominated by the ~45 MB/s (each direction,
    full duplex) axon host<->device pipe.  So the run is split into
    NSPLIT=4 *time pieces* (1024 steps/core each) that go through a
    software pipeline: while piece i executes / downloads, piece i+1's
    inputs upload and piece i-1 post-processes on the host.  Upload,
    download, device exec and host pre/post thus all overlap.
  - Downloads are int8 with a per-output-row scale (computed on device
    via max|.| + reciprocal; f32->int8 converts round-to-nearest-even).
    That halves the dominant download stream; quantization adds ~0.7%
    relative error against a 2e-2 budget.
  - The shared constants A.T/C.T/B.T/identity are shipped once per run
    as 1/8 slices (1.8 MB) and rebuilt by a tiny AllGather program whose
    outputs feed every piece call; output buffers are device-created
    once and reused WITHOUT donation (the kernel writes every output
    element), so no host zeros ever cross the link.
  - z0 only affects output rows 0..H-1; that correction and the +mean
    are applied on the host, overlapped with the downloads.
"""
import numpy as np
import concourse.bacc as bacc
import concourse.mybir as mybir
from concourse import tile

T = 32768
DZ = 512
DU = 256
NCORE = 8
TLOC = T // NCORE          # 4096 steps per core
NSPLIT = 4                 # time pieces in the host<->device pipeline
TLOCP = TLOC // NSPLIT     # 1024 steps per core per piece
S = 16                     # steps per chunk
BCH = TLOCP // S           # 64 chunks per core per piece
H = 128                    # halo steps (forgetting horizon)
K = H // S                 # 8 banded taps (incl. identity)
NCH = BCH + K              # 72 chunks in phase A
UPADP = TLOCP + H          # 1152 drive rows per core per piece

f32 = mybir.dt.float32
f32r = mybir.dt.float32r
fp16 = mybir.dt.float16
i8 = mybir.dt.int8

_CACHE = {}


def _emit_gather(nc):
    """Tiny program: AllGather the 1/8 constant slices into full copies."""
    byp = mybir.AluOpType.bypass
    rg = [list(range(NCORE))]
    with tile.TileContext(nc) as tc:
        with tc.tile_pool(name="dram", bufs=1, space="DRAM") as drampool:
            for nm, shape, dty in (
                    ("at", (DZ, DZ), f32r),
                    ("ct", (DU, DZ), fp16),
                    ("bt", (DZ, DZ), fp16),
                    ("id", (128, 128), f32)):
                din = nc.dram_tensor(nm + "s", (shape[0] // NCORE, shape[1]),
                                     dty, kind="ExternalInput")
                dout = nc.dram_tensor(nm + "g", shape, dty,
                                      kind="ExternalOutput")
                bi = drampool.tile([shape[0] // NCORE, shape[1]], dty,
                                   tag=f"agi_{nm}", name=f"agi_{nm}")
                bo = drampool.tile(list(shape), dty, tag=f"ago_{nm}",
                                   name=f"ago_{nm}")
                nc.gpsimd.dma_start(bi[:], din[:])
                nc.gpsimd.collective_compute(
                    "AllGather", byp, replica_groups=rg,
                    ins=[bi.opt()], outs=[bo.opt()])
                nc.gpsimd.dma_start(dout[:], bo[:])
    nc.compile()
    return nc


def _emit_piece(nc):
    """One pipeline piece: TLOCP steps per core, int8 output + row scales."""
    ut_d = nc.dram_tensor("ut", (DU, UPADP), fp16, kind="ExternalInput")
    at_d = nc.dram_tensor("at", (DZ, DZ), f32r, kind="ExternalInput")
    ct_d = nc.dram_tensor("ct", (DU, DZ), fp16, kind="ExternalInput")
    bt_d = nc.dram_tensor("bt", (DZ, DZ), fp16, kind="ExternalInput")
    id_d = nc.dram_tensor("id", (128, 128), f32, kind="ExternalInput")
    out_d = nc.dram_tensor("o8", (TLOCP, DZ), i8, kind="ExternalOutput")
    scl_d = nc.dram_tensor("scl", (BCH, S), fp16, kind="ExternalOutput")

    with tile.TileContext(nc) as tc:
        with tc.tile_pool(name="const", bufs=1) as cpool, \
             tc.tile_pool(name="dt", bufs=1) as dpool, \
             tc.tile_pool(name="st", bufs=2) as stpool, \
             tc.tile_pool(name="ob", bufs=4) as opool, \
             tc.tile_pool(name="ps", bufs=8, space="PSUM") as pp:

            # ---- constant loads ----
            at_sb = [cpool.tile([128, DZ], f32r, tag=f"at{k}", name=f"at{k}") for k in range(4)]
            ct_sb = [cpool.tile([128, DZ], fp16, tag=f"ct{k}", name=f"ct{k}") for k in range(2)]
            bth = [cpool.tile([128, DZ], fp16, tag=f"bth{k}", name=f"bth{k}") for k in range(4)]
            bt_sb = [cpool.tile([128, DZ], f32r, tag=f"bt{k}", name=f"bt{k}") for k in range(4)]
            id_sb = cpool.tile([128, 128], f32, tag="id")
            ut_sb = [dpool.tile([128, UPADP], fp16, tag=f"ut{k}", name=f"ut{k}") for k in range(2)]
            for k in range(4):
                nc.sync.dma_start(at_sb[k][:], at_d[128 * k:128 * (k + 1), :])
                nc.sync.dma_start(bth[k][:], bt_d[128 * k:128 * (k + 1), :])
            for k in range(2):
                nc.sync.dma_start(ct_sb[k][:], ct_d[128 * k:128 * (k + 1), :])
                nc.sync.dma_start(ut_sb[k][:], ut_d[128 * k:128 * (k + 1), :])
            nc.sync.dma_start(id_sb[:], id_d[:])
            for k in range(4):
                nc.vector.tensor_copy(bt_sb[k][:], bth[k][:])   # fp16 -> f32

            # drive rows (transposed): dt[m] holds drive.T[128m:128(m+1), :]
            dt_sb = [dpool.tile([128, UPADP], f32, tag=f"dt{m}", name=f"dt{m}") for m in range(4)]
            for nb in range((UPADP + 511) // 512):
                nb0 = nb * 512
                w = min(512, UPADP - nb0)
                for m in range(4):
                    psd = pp.tile([128, 512], f32, tag="ps", name=f"psD{nb}_{m}")
                    for kk in range(2):
                        nc.tensor.matmul(
                            psd[:, :w],
                            ct_sb[kk][:, 128 * m:128 * (m + 1)],
                            ut_sb[kk][:, nb0:nb0 + w],
                            start=(kk == 0), stop=(kk == 1))
                    nc.any.tensor_copy(dt_sb[m][:, nb0:nb0 + w], psd[:, :w])

            # ---- phase A: zero-init scan over NCH chunks ----
            bmat = [cpool.tile([128, NCH], f32r, tag=f"bm{m}", name=f"bm{m}") for m in range(4)]
            st_prev = []
            for m in range(4):
                t0 = stpool.tile([128, NCH], f32r, tag=f"st{m}", name=f"st0_{m}")
                nc.vector.tensor_copy(t0[:], dt_sb[m][:, 0:16 * NCH:16])
                st_prev.append(t0)
            for k in range(1, S):
                psl = [pp.tile([128, NCH], f32, tag="ps", name=f"psA{k}_{_m}") for _m in range(4)]
                for m in range(4):
                    for kk in range(4):
                        nc.tensor.matmul(
                            psl[m][:],
                            at_sb[kk][:, 128 * m:128 * (m + 1)],
                            st_prev[kk][:],
                            start=(kk == 0), stop=(kk == 3))
                st_new = []
                for m in range(4):
                    dst = (bmat[m] if k == S - 1 else
                           stpool.tile([128, NCH], f32r, tag=f"st{m}", name=f"stA{k}_{m}"))
                    nc.vector.tensor_tensor(
                        dst[:], psl[m][:],
                        dt_sb[m][:, k:k + 16 * (NCH - 1) + 1:16],
                        op=mybir.AluOpType.add)
                    st_new.append(dst)
                st_prev = st_new

            # ---- device-side tap matrices: G^(16p), G = A.T, via squaring ----
            def mat_t(src, dst, tg):      # dst = src.T
                for k in range(4):
                    for m in range(4):
                        pst = pp.tile([128, 128], f32, tag="ps", name=f"pT{tg}_{k}_{m}")
                        nc.tensor.transpose(
                            pst[:], src[k][:, 128 * m:128 * (m + 1)].bitcast(f32), id_sb[:])
                        nc.any.tensor_copy(dst[m][:, 128 * k:128 * (k + 1)], pst[:])

            def mat_mul(xT, y, dst, tg):  # dst = X @ Y  (xT = row-tiles of X.T)
                for m in range(4):
                    ps = pp.tile([128, DZ], f32, tag="ps", name=f"pM{tg}_{m}")
                    for k in range(4):
                        nc.tensor.matmul(
                            ps[:],
                            xT[k][:, 128 * m:128 * (m + 1)],
                            y[k][:],
                            start=(k == 0), stop=(k == 3))
                    nc.any.tensor_copy(dst[m][:], ps[:])

            px = [cpool.tile([128, DZ], f32r, tag=f"px{m}", name=f"px{m}") for m in range(4)]
            py = [cpool.tile([128, DZ], f32r, tag=f"py{m}", name=f"py{m}") for m in range(4)]
            pz = [cpool.tile([128, DZ], f32r, tag=f"pz{m}", name=f"pz{m}") for m in range(4)]

            mat_t(at_sb, px, "a")          # px = A row-tiles (= G.T)
            mat_mul(px, at_sb, py, "g2")   # py = G^2
            mat_t(py, px, "t2")
            mat_mul(px, py, pz, "g4")      # pz = G^4
            mat_t(pz, px, "t4")
            mat_mul(px, pz, py, "g8")      # py = G^8
            mat_t(py, px, "t8")
            mat_mul(px, py, pz, "g16")     # pz = G^16 (kept for the chain)

            # ---- phase B: banded combine  w_c = sum_p (A^16)^p b_{c-1-p} ----
            w_prev = []
            for m in range(4):
                wt = stpool.tile([128, BCH], f32r, tag=f"w{m}", name=f"w0_{m}")
                nc.vector.tensor_copy(wt[:], bmat[m][:, K - 1:K - 1 + BCH].bitcast(f32))
                w_prev.append(wt)
            pcur = pz
            for p in range(1, K):
                if p > 1:
                    mat_t(pcur, px, f"tp{p}")
                    mat_mul(px, pz, py, f"pp{p}")
                    pcur = py
                lo = K - 1 - p
                w_new = []
                for m in range(4):
                    ps = pp.tile([128, BCH], f32, tag="ps", name=f"psW{p}_{m}")
                    for kk in range(4):
                        nc.tensor.matmul(
                            ps[:],
                            pcur[kk][:, 128 * m:128 * (m + 1)],
                            bmat[kk][:, lo:lo + BCH],
                            start=(kk == 0), stop=(kk == 3))
                    wt = stpool.tile([128, BCH], f32r, tag=f"w{m}", name=f"w{p}_{m}")
                    nc.vector.tensor_tensor(
                        wt[:], w_prev[m][:].bitcast(f32), ps[:], op=mybir.AluOpType.add)
                    w_new.append(wt)
                w_prev = w_new

            # ---- phase C: scan BCH chunks from w_c, fused output proj ----
            # output rows t = 16*c + k quantized to int8 with per-row scale
            scl_sb = dpool.tile([BCH, S], fp16, tag="scl", name="scl")
            st_prev = w_prev
            for k in range(S):
                psl = [pp.tile([128, BCH], f32, tag="ps", name=f"psC{k}_{_m}") for _m in range(4)]
                for m in range(4):
                    for kk in range(4):
                        nc.tensor.matmul(
                            psl[m][:],
                            at_sb[kk][:, 128 * m:128 * (m + 1)],
                            st_prev[kk][:],
                            start=(kk == 0), stop=(kk == 3))
                st_new = []
                for m in range(4):
                    dst = stpool.tile([128, BCH], f32r, tag=f"sc{m}", name=f"stC{k}_{m}")
                    nc.vector.tensor_tensor(
                        dst[:], psl[m][:],
                        dt_sb[m][:, H + k:H + k + 16 * (BCH - 1) + 1:16],
                        op=mybir.AluOpType.add)
                    st_new.append(dst)
                st_prev = st_new
                pso = pp.tile([BCH, DZ], f32, tag="ps", name=f"psO{k}")
                for kk in range(4):
                    nc.tensor.matmul(
                        pso[:],
                        st_new[kk][:, 0:BCH],
                        bt_sb[kk][:],
                        start=(kk == 0), stop=(kk == 3))
                mx = opool.tile([BCH, 1], f32, tag="mx", name=f"mx{k}")
                nc.vector.tensor_reduce(
                    mx[:], pso[:], axis=mybir.AxisListType.X,
                    op=mybir.AluOpType.max, apply_absolute_value=True)
                rc = opool.tile([BCH, 1], f32, tag="rc", name=f"rc{k}")
                nc.vector.reciprocal(rc[:], mx[:])
                ob = opool.tile([BCH, DZ], i8, tag="ob", name=f"ob{k}")
                nc.vector.tensor_scalar(
                    ob[:], pso[:], rc[:], 127.0,
                    op0=mybir.AluOpType.mult, op1=mybir.AluOpType.mult)
                nc.any.tensor_copy(scl_sb[:, k:k + 1], mx[:])
                nc.sync.dma_start(out_d[k:k + 16 * (BCH - 1) + 1:16, :], ob[:])
            nc.sync.dma_start(scl_d[:], scl_sb[:])
    nc.compile()
    return nc


def _wrap_exec(nc):
    """AOT-compile the sharded executable for one bass program."""
    import jax
    from jax.sharding import Mesh, PartitionSpec, NamedSharding
    from jax.experimental.shard_map import shard_map
    from concourse import bass2jax as b2j

    partition_name = nc.partition_id_tensor.name if nc.partition_id_tensor else None
    in_names, out_names, out_avals = [], [], []
    for alloc in nc.m.functions[0].allocations:
        if not isinstance(alloc, mybir.MemoryLocationSet):
            continue
        name = alloc.memorylocations[0].name
        if alloc.kind == "ExternalInput":
            if name != partition_name:
                in_names.append(name)
        elif alloc.kind == "ExternalOutput":
            shape = tuple(alloc.tensor_shape)
            dtype = mybir.dt.np(alloc.dtype)
            out_names.append(name)
            out_avals.append(jax.core.ShapedArray(shape, dtype))
    n_params = len(in_names)
    all_in = tuple(in_names + out_names + ([partition_name] if partition_name else []))

    def _body(*args):
        operands = list(args)
        if partition_name:
            operands.append(b2j.partition_id_tensor())
        outs = b2j._bass_exec_p.bind(
            *operands,
            out_avals=tuple(out_avals),
            in_names=all_in,
            out_names=tuple(out_names),
            lowering_input_output_aliases=(),
            sim_require_finite=True,
            sim_require_nnan=True,
            nc=nc,
        )
        return tuple(outs)

    devices = jax.devices()[:NCORE]
    mesh = Mesh(np.asarray(devices), ("core",))
    shd = NamedSharding(mesh, PartitionSpec("core"))
    sharded = jax.jit(
        shard_map(
            _body, mesh=mesh,
            in_specs=(PartitionSpec("core"),) * (n_params + len(out_names)),
            out_specs=(PartitionSpec("core"),) * len(out_names),
            check_rep=False),
        keep_unused=True)
    dbg_name = nc.dbg_addr.name if nc.dbg_addr is not None else None
    return {"sharded": sharded, "in_names": in_names, "out_names": out_names,
            "out_avals": out_avals, "mesh": mesh, "shd": shd,
            "dbg_name": dbg_name}


def _build():
    if "pieces" not in _CACHE:
        b2jmod = __import__("concourse.bass2jax", fromlist=["install_neuronx_cc_hook"])
        b2jmod.install_neuronx_cc_hook()
        ncp = bacc.Bacc("TRN2", target_bir_lowering=False, debug=False,
                        num_devices=NCORE)
        _CACHE["piece_nc"] = _emit_piece(ncp)
        ncg = bacc.Bacc("TRN2", target_bir_lowering=False, debug=False,
                        num_devices=NCORE)
        _CACHE["gather_nc"] = _emit_gather(ncg)
        _CACHE["pieces"] = True
    return _CACHE["piece_nc"], _CACHE["gather_nc"]


def _make_exec():
    import functools
    import jax
    import jax.numpy as jnp

    ncp, ncg = _build()
    pe = _wrap_exec(ncp)
    ge = _wrap_exec(ncg)
    shd = pe["shd"]

    # persistent device-side output placeholder buffers (never donated; the
    # kernel writes every output element, so stale contents are harmless and
    # no host zeros ever cross the link)
    def _mkbuf(a):
        return jax.jit(functools.partial(
            jnp.zeros, (NCORE * a.shape[0],) + tuple(a.shape[1:]), a.dtype),
            out_shardings=shd)()
    pe["obufs"] = [_mkbuf(a) for a in pe["out_avals"]]
    ge["obufs"] = [_mkbuf(a) for a in ge["out_avals"]]
    for e in (pe, ge):
        if e["dbg_name"] is not None:
            e["dbgbuf"] = jax.device_put(
                np.zeros((NCORE, 2), np.uint32), shd)

    # reusable pinned host staging buffers for the per-piece uploads
    ut_bufs = [np.zeros((NCORE * DU, UPADP), np.float16) for _ in range(NSPLIT)]
    return {"pe": pe, "ge": ge, "shd": shd, "ut_bufs": ut_bufs}


def _get_state():
    if "exec" not in _CACHE:
        _CACHE["exec"] = _make_exec()
    return _CACHE["exec"]


def _correction(data, mean, A, B, R):
    """Output rows 0..H-1 need the A^n z0 contribution."""
    z0 = R.astype(np.float64) @ (data[0] - mean[0]).astype(np.float64)
    zc = z0
    A64, B64 = A.astype(np.float64), B.astype(np.float64)
    corr = np.empty((H, DZ), np.float64)
    for n in range(1, H + 1):
        zc = A64 @ zc
        corr[n - 1] = B64 @ zc
    return corr.astype(np.float32)


def _invoke(state, data, inputs_np, mean, A, B, C, R):
    """One full pipelined run."""
    import jax
    pe, ge, shd = state["pe"], state["ge"], state["shd"]

    # ---- constants: ship 1/8 slices, AllGather on device ----
    atT = np.ascontiguousarray(A.T, dtype=np.float32)
    ctT = np.ascontiguousarray(C.T).astype(np.float16)
    btT = np.ascontiguousarray(B.T).astype(np.float16)
    idm = np.eye(128, dtype=np.float32)
    gfeed = {"ats": atT, "cts": ctT, "bts": btT, "ids": idm}
    gargs = [jax.device_put(gfeed[n], shd) for n in ge["in_names"]]
    gall = list(gargs) + list(ge["obufs"])
    if ge["dbg_name"] is not None:
        gall.append(ge["dbgbuf"])
    gouts = ge["sharded"](*gall)

    # map gather outputs to piece inputs by name ("atg" -> "at", ...)
    gmap = {n: o for n, o in zip(ge["out_names"], gouts)}
    consts = {nm: gmap[nm + "g"] for nm in ("at", "ct", "bt", "id")}

    # ---- pipelined pieces ----
    uT = inputs_np.T  # (DU, T) strided view; the per-piece astype does the copy
    piece_outs = []
    for i in range(NSPLIT):
        buf = state["ut_bufs"][i]
        for j in range(NCORE):
            lo = j * TLOC + i * TLOCP - H
            if lo < 0:  # piece 0, core 0: zero history
                buf[j * DU:(j + 1) * DU, :H] = 0.0
                buf[j * DU:(j + 1) * DU, H:] = uT[:, 0:TLOCP]
            else:
                buf[j * DU:(j + 1) * DU, :] = uT[:, lo:lo + UPADP]
        uti = jax.device_put(buf, shd)
        pargs = []
        for n in pe["in_names"]:
            pargs.append(uti if n == "ut" else consts[n])
        pall = pargs + list(pe["obufs"])
        if pe["dbg_name"] is not None:
            pall.append(pe["dbgbuf"])
        outs = pe["sharded"](*pall)
        for o in outs:
            try:
                o.copy_to_host_async()
            except Exception:
                pass
        piece_outs.append(outs)

    # ---- host post, overlapped with the downloads ----
    corr = _correction(data, mean, A, B, R)
    omap = {n: k for k, n in enumerate(pe["out_names"])}
    out = np.empty((T, DZ), np.float32)
    for i in range(NSPLIT):
        outs = piece_outs[i]
        o8 = np.asarray(outs[omap["o8"]])          # (NCORE*TLOCP, DZ) int8
        scl = np.asarray(outs[omap["scl"]])        # (NCORE*BCH, S) fp16
        for j in range(NCORE):
            s = scl[j * BCH:(j + 1) * BCH].astype(np.float32).reshape(-1)
            s *= (1.0 / 127.0)
            view = out[j * TLOC + i * TLOCP: j * TLOC + (i + 1) * TLOCP]
            np.multiply(o8[j * TLOCP:(j + 1) * TLOCP], s[:, None], out=view)
            np.add(view, mean, out=view)
    out[:H] += corr
    return out


def kernel(data, inputs, mean, A, B, C, recognition_matrix, steps=None, **kw):
    data = np.asarray(data, np.float32)
    inputs_np = np.asarray(inputs, np.float32)
    mean = np.asarray(mean, np.float32)
    A = np.asarray(A, np.float32)
    B = np.asarray(B, np.float32)
    C = np.asarray(C, np.float32)
    R = np.asarray(recognition_matrix, np.float32)

    state = _get_state()
    return _invoke(state, data, inputs_np, mean, A, B, C, R)


# revision 4
# speedup vs baseline: 2.1586x; 1.3096x over previous
"""Trainium2 Bass kernel for the KalmanFilter linear recurrence.

  x = data - mean;  z0 = R @ x[0];  drive = inputs @ C.T
  z_{t+1} = A z_t + drive[t]   (T = 32768 steps, dim 512)
  result  = Z[1:] @ B.T + mean

Strategy (8 NeuronCores, sequence-parallel + host<->device pipelining):
  - ||A^k|| decays like 0.9^k, so the recurrence forgets its state after
    H=128 steps.  Each core owns 4096 contiguous steps, processed per
    128-step-halo'd block with the classic blocked-scan trio:
      Phase A: zero-init scan over 16-step chunks -> accumulated drives
      Phase B: banded combine with on-device (A^16)^p tap matrices
      Phase C: re-scan from combined inits, fused output projection B.T
  - The end-to-end time is dominated by the ~25-45 MB/s (each direction,
    full duplex) axon host<->device pipe with a large per-transfer fixed
    cost.  So: (a) inputs ship as uint8 (offset-128 int8, global scale
    su = 127/max|u|; the whole pipeline is linear, so the su factor is
    divided out of the downloaded row scales on the host); (b) outputs
    ship as int8 with a per-row fp16 scale packed into 2 extra columns
    (device computes row max|.| + reciprocal; f32->int8 converts RNE);
    (c) uploads happen in 2 halves while execution is split into 4 time
    pieces via two program variants that read the same uploaded buffer
    at different column offsets - so upload, exec, download and host
    pre/post all overlap in a software pipeline.
  - The shared constants A.T/C.T/B.T/identity ship once per run as 1/8
    slices and a tiny AllGather program rebuilds full copies that feed
    every piece call; output buffers are device-created once and reused
    WITHOUT donation (the kernel writes every output element), so no
    host zeros ever cross the link.
  - z0 only affects output rows 0..H-1; that correction and the +mean
    are applied on the host, overlapped with the downloads.
"""
import numpy as np
import concourse.bacc as bacc
import concourse.mybir as mybir
from concourse import tile

T = 32768
DZ = 512
DU = 256
NCORE = 8
TLOC = T // NCORE          # 4096 steps per core
NSPLIT = 4                 # exec/download pieces
NUP = 2                    # upload halves (2 pieces per half)
TLOCP = TLOC // NSPLIT     # 1024 steps per core per piece
S = 16                     # steps per chunk
BCH = TLOCP // S           # 64 chunks per core per piece
H = 128                    # halo steps (forgetting horizon)
K = H // S                 # 8 banded taps (incl. identity)
NCH = BCH + K              # 72 chunks in phase A
UPADP = TLOCP + H          # 1152 drive rows read per piece
UHALF = 2 * TLOCP + H      # 2176 drive rows uploaded per half

f32 = mybir.dt.float32
f32r = mybir.dt.float32r
fp16 = mybir.dt.float16
i8 = mybir.dt.int8
u8 = mybir.dt.uint8

_CACHE = {}


def _emit_gather(nc):
    """Tiny program: AllGather the 1/8 constant slices into full copies."""
    byp = mybir.AluOpType.bypass
    rg = [list(range(NCORE))]
    with tile.TileContext(nc) as tc:
        with tc.tile_pool(name="dram", bufs=1, space="DRAM") as drampool:
            for nm, shape, dty in (
                    ("at", (DZ, DZ), f32r),
                    ("ct", (DU, DZ), fp16),
                    ("bt", (DZ, DZ), fp16),
                    ("id", (128, 128), f32)):
                din = nc.dram_tensor(nm + "s", (shape[0] // NCORE, shape[1]),
                                     dty, kind="ExternalInput")
                dout = nc.dram_tensor(nm + "g", shape, dty,
                                      kind="ExternalOutput")
                bi = drampool.tile([shape[0] // NCORE, shape[1]], dty,
                                   tag=f"agi_{nm}", name=f"agi_{nm}")
                bo = drampool.tile(list(shape), dty, tag=f"ago_{nm}",
                                   name=f"ago_{nm}")
                nc.gpsimd.dma_start(bi[:], din[:])
                nc.gpsimd.collective_compute(
                    "AllGather", byp, replica_groups=rg,
                    ins=[bi.opt()], outs=[bo.opt()])
                nc.gpsimd.dma_start(dout[:], bo[:])
    nc.compile()
    return nc


def _emit_piece(nc, ofs):
    """One pipeline piece: TLOCP steps/core starting at column `ofs` of the
    uploaded uint8 half-buffer.  Output int8 + per-row fp16 scale in cols
    512:514."""
    ut_d = nc.dram_tensor("u8", (DU, UHALF), u8, kind="ExternalInput")
    at_d = nc.dram_tensor("at", (DZ, DZ), f32r, kind="ExternalInput")
    ct_d = nc.dram_tensor("ct", (DU, DZ), fp16, kind="ExternalInput")
    bt_d = nc.dram_tensor("bt", (DZ, DZ), fp16, kind="ExternalInput")
    id_d = nc.dram_tensor("id", (128, 128), f32, kind="ExternalInput")
    out_d = nc.dram_tensor("o8", (TLOCP, DZ + 2), i8, kind="ExternalOutput")

    with tile.TileContext(nc) as tc:
        with tc.tile_pool(name="const", bufs=1) as cpool, \
             tc.tile_pool(name="dt", bufs=1) as dpool, \
             tc.tile_pool(name="st", bufs=2) as stpool, \
             tc.tile_pool(name="ob", bufs=4) as opool, \
             tc.tile_pool(name="ps", bufs=8, space="PSUM") as pp:

            # ---- constant loads ----
            at_sb = [cpool.tile([128, DZ], f32r, tag=f"at{k}", name=f"at{k}") for k in range(4)]
            ct_sb = [cpool.tile([128, DZ], fp16, tag=f"ct{k}", name=f"ct{k}") for k in range(2)]
            bth = [cpool.tile([128, DZ], fp16, tag=f"bth{k}", name=f"bth{k}") for k in range(4)]
            bt_sb = [cpool.tile([128, DZ], f32r, tag=f"bt{k}", name=f"bt{k}") for k in range(4)]
            id_sb = cpool.tile([128, 128], f32, tag="id")
            ut8_sb = [dpool.tile([128, UPADP], u8, tag=f"u8{k}", name=f"u8{k}") for k in range(2)]
            ut_sb = [dpool.tile([128, UPADP], fp16, tag=f"ut{k}", name=f"ut{k}") for k in range(2)]
            for k in range(4):
                nc.sync.dma_start(at_sb[k][:], at_d[128 * k:128 * (k + 1), :])
                nc.sync.dma_start(bth[k][:], bt_d[128 * k:128 * (k + 1), :])
            for k in range(2):
                nc.sync.dma_start(ct_sb[k][:], ct_d[128 * k:128 * (k + 1), :])
                nc.sync.dma_start(ut8_sb[k][:], ut_d[128 * k:128 * (k + 1), ofs:ofs + UPADP])
            nc.sync.dma_start(id_sb[:], id_d[:])
            for k in range(4):
                nc.vector.tensor_copy(bt_sb[k][:], bth[k][:])   # fp16 -> f32
            for k in range(2):                                  # uint8 -> fp16, -128
                nc.vector.tensor_scalar(
                    ut_sb[k][:], ut8_sb[k][:], 128.0, None,
                    op0=mybir.AluOpType.subtract)

            # drive rows (transposed): dt[m] holds drive.T[128m:128(m+1), :]
            dt_sb = [dpool.tile([128, UPADP], f32, tag=f"dt{m}", name=f"dt{m}") for m in range(4)]
            for nb in range((UPADP + 511) // 512):
                nb0 = nb * 512
                w = min(512, UPADP - nb0)
                for m in range(4):
                    psd = pp.tile([128, 512], f32, tag="ps", name=f"psD{nb}_{m}")
                    for kk in range(2):
                        nc.tensor.matmul(
                            psd[:, :w],
                            ct_sb[kk][:, 128 * m:128 * (m + 1)],
                            ut_sb[kk][:, nb0:nb0 + w],
                            start=(kk == 0), stop=(kk == 1))
                    nc.any.tensor_copy(dt_sb[m][:, nb0:nb0 + w], psd[:, :w])

            # ---- phase A: zero-init scan over NCH chunks ----
            bmat = [cpool.tile([128, NCH], f32r, tag=f"bm{m}", name=f"bm{m}") for m in range(4)]
            st_prev = []
            for m in range(4):
                t0 = stpool.tile([128, NCH], f32r, tag=f"st{m}", name=f"st0_{m}")
                nc.vector.tensor_copy(t0[:], dt_sb[m][:, 0:16 * NCH:16])
                st_prev.append(t0)
            for k in range(1, S):
                psl = [pp.tile([128, NCH], f32, tag="ps", name=f"psA{k}_{_m}") for _m in range(4)]
                for m in range(4):
                    for kk in range(4):
                        nc.tensor.matmul(
                            psl[m][:],
                            at_sb[kk][:, 128 * m:128 * (m + 1)],
                            st_prev[kk][:],
                            start=(kk == 0), stop=(kk == 3))
                st_new = []
                for m in range(4):
                    dst = (bmat[m] if k == S - 1 else
                           stpool.tile([128, NCH], f32r, tag=f"st{m}", name=f"stA{k}_{m}"))
                    nc.vector.tensor_tensor(
                        dst[:], psl[m][:],
                        dt_sb[m][:, k:k + 16 * (NCH - 1) + 1:16],
                        op=mybir.AluOpType.add)
                    st_new.append(dst)
                st_prev = st_new

            # ---- device-side tap matrices: G^(16p), G = A.T, via squaring ----
            def mat_t(src, dst, tg):      # dst = src.T
                for k in range(4):
                    for m in range(4):
                        pst = pp.tile([128, 128], f32, tag="ps", name=f"pT{tg}_{k}_{m}")
                        nc.tensor.transpose(
                            pst[:], src[k][:, 128 * m:128 * (m + 1)].bitcast(f32), id_sb[:])
                        nc.any.tensor_copy(dst[m][:, 128 * k:128 * (k + 1)], pst[:])

            def mat_mul(xT, y, dst, tg):  # dst = X @ Y  (xT = row-tiles of X.T)
                for m in range(4):
                    ps = pp.tile([128, DZ], f32, tag="ps", name=f"pM{tg}_{m}")
                    for k in range(4):
                        nc.tensor.matmul(
                            ps[:],
                            xT[k][:, 128 * m:128 * (m + 1)],
                            y[k][:],
                            start=(k == 0), stop=(k == 3))
                    nc.any.tensor_copy(dst[m][:], ps[:])

            px = [cpool.tile([128, DZ], f32r, tag=f"px{m}", name=f"px{m}") for m in range(4)]
            py = [cpool.tile([128, DZ], f32r, tag=f"py{m}", name=f"py{m}") for m in range(4)]
            pz = [cpool.tile([128, DZ], f32r, tag=f"pz{m}", name=f"pz{m}") for m in range(4)]

            mat_t(at_sb, px, "a")          # px = A row-tiles (= G.T)
            mat_mul(px, at_sb, py, "g2")   # py = G^2
            mat_t(py, px, "t2")
            mat_mul(px, py, pz, "g4")      # pz = G^4
            mat_t(pz, px, "t4")
            mat_mul(px, pz, py, "g8")      # py = G^8
            mat_t(py, px, "t8")
            mat_mul(px, py, pz, "g16")     # pz = G^16 (kept for the chain)

            # ---- phase B: banded combine  w_c = sum_p (A^16)^p b_{c-1-p} ----
            w_prev = []
            for m in range(4):
                wt = stpool.tile([128, BCH], f32r, tag=f"w{m}", name=f"w0_{m}")
                nc.vector.tensor_copy(wt[:], bmat[m][:, K - 1:K - 1 + BCH].bitcast(f32))
                w_prev.append(wt)
            pcur = pz
            for p in range(1, K):
                if p > 1:
                    mat_t(pcur, px, f"tp{p}")
                    mat_mul(px, pz, py, f"pp{p}")
                    pcur = py
                lo = K - 1 - p
                w_new = []
                for m in range(4):
                    ps = pp.tile([128, BCH], f32, tag="ps", name=f"psW{p}_{m}")
                    for kk in range(4):
                        nc.tensor.matmul(
                            ps[:],
                            pcur[kk][:, 128 * m:128 * (m + 1)],
                            bmat[kk][:, lo:lo + BCH],
                            start=(kk == 0), stop=(kk == 3))
                    wt = stpool.tile([128, BCH], f32r, tag=f"w{m}", name=f"w{p}_{m}")
                    nc.vector.tensor_tensor(
                        wt[:], w_prev[m][:].bitcast(f32), ps[:], op=mybir.AluOpType.add)
                    w_new.append(wt)
                w_prev = w_new

            # ---- phase C: scan BCH chunks from w_c, fused output proj ----
            # row t = 16*c + k: int8 payload in cols 0:512, fp16 scale in 512:514
            st_prev = w_prev
            for k in range(S):
                psl = [pp.tile([128, BCH], f32, tag="ps", name=f"psC{k}_{_m}") for _m in range(4)]
                for m in range(4):
                    for kk in range(4):
                        nc.tensor.matmul(
                            psl[m][:],
                            at_sb[kk][:, 128 * m:128 * (m + 1)],
                            st_prev[kk][:],
                            start=(kk == 0), stop=(kk == 3))
                st_new = []
                for m in range(4):
                    dst = stpool.tile([128, BCH], f32r, tag=f"sc{m}", name=f"stC{k}_{m}")
                    nc.vector.tensor_tensor(
                        dst[:], psl[m][:],
                        dt_sb[m][:, H + k:H + k + 16 * (BCH - 1) + 1:16],
                        op=mybir.AluOpType.add)
                    st_new.append(dst)
                st_prev = st_new
                pso = pp.tile([BCH, DZ], f32, tag="ps", name=f"psO{k}")
                for kk in range(4):
                    nc.tensor.matmul(
                        pso[:],
                        st_new[kk][:, 0:BCH],
                        bt_sb[kk][:],
                        start=(kk == 0), stop=(kk == 3))
                mx = opool.tile([BCH, 1], f32, tag="mx", name=f"mx{k}")
                nc.vector.tensor_reduce(
                    mx[:], pso[:], axis=mybir.AxisListType.X,
                    op=mybir.AluOpType.max, apply_absolute_value=True)
                rc = opool.tile([BCH, 1], f32, tag="rc", name=f"rc{k}")
                nc.vector.reciprocal(rc[:], mx[:])
                ob = opool.tile([BCH, DZ], i8, tag="ob", name=f"ob{k}")
                nc.vector.tensor_scalar(
                    ob[:], pso[:], rc[:], 127.0,
                    op0=mybir.AluOpType.mult, op1=mybir.AluOpType.mult)
                mxh = opool.tile([BCH, 1], fp16, tag="mh", name=f"mh{k}")
                nc.any.tensor_copy(mxh[:], mx[:])
                nc.sync.dma_start(out_d[k:k + 16 * (BCH - 1) + 1:16, 0:DZ], ob[:])
                nc.sync.dma_start(out_d[k:k + 16 * (BCH - 1) + 1:16, DZ:DZ + 2],
                                  mxh[:].bitcast(i8))
    nc.compile()
    return nc


def _wrap_exec(nc):
    """Build the sharded executable for one bass program."""
    import jax
    from jax.sharding import Mesh, PartitionSpec, NamedSharding
    from jax.experimental.shard_map import shard_map
    from concourse import bass2jax as b2j

    partition_name = nc.partition_id_tensor.name if nc.partition_id_tensor else None
    in_names, out_names, out_avals = [], [], []
    for alloc in nc.m.functions[0].allocations:
        if not isinstance(alloc, mybir.MemoryLocationSet):
            continue
        name = alloc.memorylocations[0].name
        if alloc.kind == "ExternalInput":
            if name != partition_name:
                in_names.append(name)
        elif alloc.kind == "ExternalOutput":
            shape = tuple(alloc.tensor_shape)
            dtype = mybir.dt.np(alloc.dtype)
            out_names.append(name)
            out_avals.append(jax.core.ShapedArray(shape, dtype))
    n_params = len(in_names)
    all_in = tuple(in_names + out_names + ([partition_name] if partition_name else []))

    def _body(*args):
        operands = list(args)
        if partition_name:
            operands.append(b2j.partition_id_tensor())
        outs = b2j._bass_exec_p.bind(
            *operands,
            out_avals=tuple(out_avals),
            in_names=all_in,
            out_names=tuple(out_names),
            lowering_input_output_aliases=(),
            sim_require_finite=True,
            sim_require_nnan=True,
            nc=nc,
        )
        return tuple(outs)

    devices = jax.devices()[:NCORE]
    mesh = Mesh(np.asarray(devices), ("core",))
    shd = NamedSharding(mesh, PartitionSpec("core"))
    sharded = jax.jit(
        shard_map(
            _body, mesh=mesh,
            in_specs=(PartitionSpec("core"),) * (n_params + len(out_names)),
            out_specs=(PartitionSpec("core"),) * len(out_names),
            check_rep=False),
        keep_unused=True)
    dbg_name = nc.dbg_addr.name if nc.dbg_addr is not None else None
    return {"sharded": sharded, "in_names": in_names, "out_names": out_names,
            "out_avals": out_avals, "mesh": mesh, "shd": shd,
            "dbg_name": dbg_name}


def _build():
    if "piece_ncs" not in _CACHE:
        b2jmod = __import__("concourse.bass2jax", fromlist=["install_neuronx_cc_hook"])
        b2jmod.install_neuronx_cc_hook()
        piece_ncs = []
        for v in range(2):
            ncp = bacc.Bacc("TRN2", target_bir_lowering=False, debug=False,
                            num_devices=NCORE)
            piece_ncs.append(_emit_piece(ncp, v * TLOCP))
        ncg = bacc.Bacc("TRN2", target_bir_lowering=False, debug=False,
                        num_devices=NCORE)
        _CACHE["piece_ncs"] = piece_ncs
        _CACHE["gather_nc"] = _emit_gather(ncg)
    return _CACHE["piece_ncs"], _CACHE["gather_nc"]


def _make_exec():
    import functools
    import jax
    import jax.numpy as jnp

    piece_ncs, ncg = _build()
    pes = [_wrap_exec(ncp) for ncp in piece_ncs]
    ge = _wrap_exec(ncg)
    shd = pes[0]["shd"]

    def _mkbuf(a):
        return jax.jit(functools.partial(
            jnp.zeros, (NCORE * a.shape[0],) + tuple(a.shape[1:]), a.dtype),
            out_shardings=shd)()
    obufs = [_mkbuf(a) for a in pes[0]["out_avals"]]
    for pe in pes:
        pe["obufs"] = obufs
    ge["obufs"] = [_mkbuf(a) for a in ge["out_avals"]]
    for e in (*pes, ge):
        if e["dbg_name"] is not None:
            e["dbgbuf"] = jax.device_put(
                np.zeros((NCORE, 2), np.uint32), shd)

    fbufs = [np.zeros((NCORE * DU, UHALF), np.float32) for _ in range(NUP)]
    ubufs = [np.zeros((NCORE * DU, UHALF), np.uint8) for _ in range(NUP)]
    return {"pes": pes, "ge": ge, "shd": shd, "fbufs": fbufs, "ubufs": ubufs}


def _get_state():
    if "exec" not in _CACHE:
        _CACHE["exec"] = _make_exec()
    return _CACHE["exec"]


def _correction(data, mean, A, B, R):
    """Output rows 0..H-1 need the A^n z0 contribution."""
    z0 = R.astype(np.float64) @ (data[0] - mean[0]).astype(np.float64)
    zc = z0
    A64, B64 = A.astype(np.float64), B.astype(np.float64)
    corr = np.empty((H, DZ), np.float64)
    for n in range(1, H + 1):
        zc = A64 @ zc
        corr[n - 1] = B64 @ zc
    return corr.astype(np.float32)


def _invoke(state, data, inputs_np, mean, A, B, C, R):
    """One full pipelined run."""
    import jax
    pes, ge, shd = state["pes"], state["ge"], state["shd"]

    # ---- constants: ship 1/8 slices, AllGather on device ----
    atT = np.ascontiguousarray(A.T, dtype=np.float32)
    ctT = np.ascontiguousarray(C.T).astype(np.float16)
    btT = np.ascontiguousarray(B.T).astype(np.float16)
    idm = np.eye(128, dtype=np.float32)
    gfeed = {"ats": atT, "cts": ctT, "bts": btT, "ids": idm}
    gargs = [jax.device_put(gfeed[n], shd) for n in ge["in_names"]]
    gall = list(gargs) + list(ge["obufs"])
    if ge["dbg_name"] is not None:
        gall.append(ge["dbgbuf"])
    gouts = ge["sharded"](*gall)
    gmap = {n: o for n, o in zip(ge["out_names"], gouts)}
    consts = {nm: gmap[nm + "g"] for nm in ("at", "ct", "bt", "id")}

    # ---- input quantization scale ----
    amax = float(np.abs(inputs_np).max())
    su = 127.0 / amax if amax > 0 else 1.0
    uT = inputs_np.T  # (DU, T) strided view

    # ---- pipelined halves / pieces ----
    piece_outs = []
    for h in range(NUP):
        fbuf, ubuf = state["fbufs"][h], state["ubufs"][h]
        for j in range(NCORE):
            lo = j * TLOC + h * (NUP * TLOCP) - H
            tgt = fbuf[j * DU:(j + 1) * DU]
            if lo < 0:
                tgt[:, :H] = 0.0
                tgt[:, H:] = uT[:, 0:UHALF - H]
            else:
                tgt[:] = uT[:, lo:lo + UHALF]
        np.multiply(fbuf, su, out=fbuf)
        np.add(fbuf, 128.5, out=fbuf)
        ubuf[:] = fbuf  # trunc-toward-zero == floor for positive values
        uh = jax.device_put(ubuf, shd)
        for v in range(2):
            pe = pes[v]
            pargs = [uh if n == "u8" else consts[n] for n in pe["in_names"]]
            pall = pargs + list(pe["obufs"])
            if pe["dbg_name"] is not None:
                pall.append(pe["dbgbuf"])
            outs = pe["sharded"](*pall)
            for o in outs:
                try:
                    o.copy_to_host_async()
                except Exception:
                    pass
            piece_outs.append(outs)

    # ---- host post, overlapped with the downloads ----
    corr = _correction(data, mean, A, B, R)
    dq = 1.0 / (127.0 * su)
    out = np.empty((T, DZ), np.float32)
    for i in range(NSPLIT):
        o8 = np.asarray(piece_outs[i][0])          # (NCORE*TLOCP, DZ+2) int8
        for j in range(NCORE):
            blk = o8[j * TLOCP:(j + 1) * TLOCP]
            s = np.ascontiguousarray(blk[:, DZ:DZ + 2]).view(np.float16)
            s = s.astype(np.float32).reshape(-1)
            s *= dq
            view = out[j * TLOC + i * TLOCP: j * TLOC + (i + 1) * TLOCP]
            np.multiply(blk[:, :DZ], s[:, None], out=view)
            np.add(view, mean, out=view)
    out[:H] += corr
    return out


def kernel(data, inputs, mean, A, B, C, recognition_matrix, steps=None, **kw):
    data = np.asarray(data, np.float32)
    inputs_np = np.asarray(inputs, np.float32)
    mean = np.asarray(mean, np.float32)
    A = np.asarray(A, np.float32)
    B = np.asarray(B, np.float32)
    C = np.asarray(C, np.float32)
    R = np.asarray(recognition_matrix, np.float32)

    state = _get_state()
    return _invoke(state, data, inputs_np, mean, A, B, C, R)
